# revision 2
# baseline (speedup 1.0000x reference)
"""Trainium2 Bass kernel for nn_ContextAttention_21457656611319.

Reference math (per batch n):
    xf = x[n] reshaped [C, L], L = H*W = 4096
    q = Wq@xf + bq ; k = Wk@xf + bk ; v = Wv@xf + bv          [C, L]
    S[l,m] = sum_c k[c,l] q[c,m] * (1/sqrt(C))                 [L, L]
    T = softmax(S, axis=m)  (softmax over the m axis)
    attn[c,m] = sum_l v[c,l] T[l,m]
    out = x + attn

Sharding: 8 cores = 4 batches x 2-way shard of the l (key/value) axis.
Softmax rows (fixed l, all m) stay intact on one core, so each core
computes a partial attn (partial sum over its l-half) independently;
the host adds the two halves per batch plus x.  No collectives needed.

Per-core kernel (l-half LH=2048, 16 l-tiles of 128):
  phase 0: q = WqT^T @ xf (+bq)   [C, L]   (SBUF)
           k = WkT^T @ xh (+bk)   [C, LH]  (SBUF)
           vT_i = xh_i^T @ WvT (+bv)  [128, C] per l-tile (SBUF)
  phase 1 (per l-tile i): S_i = k_i^T @ q in PSUM (two 2048-col chunks),
           T_i = exp(scale*S_i) -> SBUF bf16 (ACT, with accumulated
           row-sum Z_i), r_i = 1/Z_i, vts_i = vT_i * r_i (bf16)
  phase 2 (per 512-col bank b): attn[:,b] = sum_i vts_i^T @ T_i[:,b]
           accumulated in PSUM, copied to SBUF, DMA'd out.

Softmax max-subtraction is skipped: scores*scale ~ N(0,1) here, so
exp() cannot overflow and softmax is shift-invariant anyway.
"""

import sys

if "/opt/trn_rl_repo" not in sys.path:
    sys.path.insert(0, "/opt/trn_rl_repo")

import numpy as np

N, C, H, W = 4, 128, 64, 64
L = H * W            # 4096
LH = L // 2          # 2048 l-half per core
P = 128              # partitions / l-tile size
NT = LH // P         # 16 l-tiles per core
BANK = 512           # fp32 elems per PSUM bank
NB = L // BANK       # 8 output column banks
NCORES = 8
SCALE = float(1.0 / np.sqrt(C))

_CACHE = {}


def _build_nc():
    import concourse.bass as bass
    import concourse.tile as tile
    from concourse import bacc, mybir
    from contextlib import ExitStack

    f32 = mybir.dt.float32
    bf16 = mybir.dt.bfloat16

    nc = bacc.Bacc("TRN2", target_bir_lowering=False, debug=False)

    xf = nc.dram_tensor("xf", [P, L], f32, kind="ExternalInput").ap()
    xh = nc.dram_tensor("xh", [P, LH], f32, kind="ExternalInput").ap()
    wqT = nc.dram_tensor("wqT", [P, P], f32, kind="ExternalInput").ap()
    wkT = nc.dram_tensor("wkT", [P, P], f32, kind="ExternalInput").ap()
    wvT = nc.dram_tensor("wvT", [P, P], f32, kind="ExternalInput").ap()
    bq = nc.dram_tensor("bq", [P, 1], f32, kind="ExternalInput").ap()
    bk = nc.dram_tensor("bk", [P, 1], f32, kind="ExternalInput").ap()
    bv = nc.dram_tensor("bv", [1, P], f32, kind="ExternalInput").ap()
    attn_out = nc.dram_tensor("attn_part", [P, L], f32, kind="ExternalOutput").ap()

    Exp = mybir.ActivationFunctionType.Exp

    with tile.TileContext(nc) as tc, ExitStack() as ctx:
        const = ctx.enter_context(tc.tile_pool(name="const", bufs=1))
        persist = ctx.enter_context(tc.tile_pool(name="persist", bufs=1))

        wq_sb = const.tile([P, P], f32)
        wk_sb = const.tile([P, P], f32)
        wv_sb = const.tile([P, P], f32)
        bq_sb = const.tile([P, 1], f32)
        bk_sb = const.tile([P, 1], f32)
        bv_sb = const.tile([P, P], f32)  # bv broadcast across partitions
        nc.sync.dma_start(out=wq_sb, in_=wqT)
        nc.sync.dma_start(out=wk_sb, in_=wkT)
        nc.sync.dma_start(out=wv_sb, in_=wvT)
        nc.sync.dma_start(out=bq_sb, in_=bq)
        nc.sync.dma_start(out=bk_sb, in_=bk)
        bv_bcast = bass.AP(tensor=bv.tensor, offset=bv.offset,
                           ap=[[0, P], bv.ap[1]])
        nc.sync.dma_start(out=bv_sb, in_=bv_bcast)

        q_sb = persist.tile([P, L], f32)
        k_sb = persist.tile([P, LH], f32)
        vt_sb = persist.tile([P, NT, P], f32)   # [l, tile, c]
        vts = persist.tile([P, NT, P], bf16)    # vT * (1/Z), bf16
        z2 = persist.tile([P, NT, 2], f32)      # per-chunk exp row sums
        zs = persist.tile([P, NT], f32)
        rs = persist.tile([P, NT], f32)

        # ---- phase 0: projections -------------------------------------
        with tc.tile_pool(name="xp", bufs=1) as xp, \
             tc.tile_pool(name="p0ps", bufs=2, space="PSUM") as p0:
            x_sb = xp.tile([P, L], f32)
            xh_sb = xp.tile([P, LH], f32)
            nc.sync.dma_start(out=x_sb, in_=xf)
            nc.sync.dma_start(out=xh_sb, in_=xh)

            # q = WqT^T @ x + bq, in two 2048-wide passes
            for h in range(2):
                t = p0.tile([P, LH], f32, tag="p0")
                for j in range(LH // BANK):
                    c0 = h * LH + j * BANK
                    nc.tensor.matmul(t[:, j * BANK:(j + 1) * BANK],
                                     wq_sb, x_sb[:, c0:c0 + BANK])
                nc.vector.tensor_scalar_add(q_sb[:, h * LH:(h + 1) * LH], t, bq_sb)

            # k = WkT^T @ xh + bk
            t = p0.tile([P, LH], f32, tag="p0")
            for j in range(LH // BANK):
                nc.tensor.matmul(t[:, j * BANK:(j + 1) * BANK],
                                 wk_sb, xh_sb[:, j * BANK:(j + 1) * BANK])
            nc.vector.tensor_scalar_add(k_sb, t, bk_sb)

            # vT_i = xh_i^T @ WvT  (+ bv broadcast along free dim)
            t = p0.tile([P, LH], f32, tag="p0")
            for i in range(NT):
                nc.tensor.matmul(t[:, i * P:(i + 1) * P],
                                 xh_sb[:, i * P:(i + 1) * P], wv_sb)
            for i in range(NT):
                nc.vector.tensor_add(vt_sb[:, i, :], t[:, i * P:(i + 1) * P], bv_sb)

        # ---- T storage (reuses the SBUF space freed by xp) -------------
        tpool = ctx.enter_context(tc.tile_pool(name="tpool", bufs=1))
        t_all = tpool.tile([P, NT, L], bf16)

        # ---- phase 1: scores + softmax numerator ----------------------
        with tc.tile_pool(name="sps", bufs=2, space="PSUM") as sp:
            for i in range(NT):
                for h in range(2):
                    s = sp.tile([P, LH], f32, tag="s")
                    for j in range(LH // BANK):
                        nc.tensor.matmul(s[:, j * BANK:(j + 1) * BANK],
                                         k_sb[:, i * P:(i + 1) * P],
                                         q_sb[:, h * LH + j * BANK:h * LH + (j + 1) * BANK])
                    nc.scalar.activation(t_all[:, i, h * LH:(h + 1) * LH], s,
                                         Exp, scale=SCALE,
                                         accum_out=z2[:, i, h:h + 1])
                nc.vector.tensor_add(zs[:, i:i + 1], z2[:, i, 0:1], z2[:, i, 1:2])
                nc.vector.reciprocal(rs[:, i:i + 1], zs[:, i:i + 1])
                nc.vector.tensor_scalar_mul(vts[:, i, :], vt_sb[:, i, :], rs[:, i:i + 1])

        # ---- phase 2: attn_part = sum_i vts_i^T @ T_i ------------------
        with tc.tile_pool(name="aps", bufs=2, space="PSUM") as ap, \
             tc.tile_pool(name="outp", bufs=2) as outp:
            for b in range(NB):
                acc = ap.tile([P, BANK], f32, tag="acc")
                for i in range(NT):
                    nc.tensor.matmul(acc, vts[:, i, :],
                                     t_all[:, i, b * BANK:(b + 1) * BANK],
                                     start=(i == 0), stop=(i == NT - 1))
                ao = outp.tile([P, BANK], f32, tag="ao")
                nc.vector.tensor_copy(ao, acc)
                nc.sync.dma_start(out=attn_out[:, b * BANK:(b + 1) * BANK], in_=ao)

    nc.compile()
    return nc


def _get_nc():
    if "nc" not in _CACHE:
        _CACHE["nc"] = _build_nc()
    return _CACHE["nc"]


def _make_in_maps(inputs):
    x = np.ascontiguousarray(np.asarray(inputs["x"], dtype=np.float32))
    wqT = np.ascontiguousarray(np.asarray(inputs["Wq"], dtype=np.float32).T)
    wkT = np.ascontiguousarray(np.asarray(inputs["Wk"], dtype=np.float32).T)
    wvT = np.ascontiguousarray(np.asarray(inputs["Wv"], dtype=np.float32).T)
    bq = np.ascontiguousarray(np.asarray(inputs["bq"], dtype=np.float32).reshape(P, 1))
    bk = np.ascontiguousarray(np.asarray(inputs["bk"], dtype=np.float32).reshape(P, 1))
    bv = np.ascontiguousarray(np.asarray(inputs["bv"], dtype=np.float32).reshape(1, P))
    in_maps = []
    for core in range(NCORES):
        n, half = core // 2, core % 2
        xf = np.ascontiguousarray(x[n].reshape(C, L))
        xh = np.ascontiguousarray(xf[:, half * LH:(half + 1) * LH])
        in_maps.append({
            "xf": xf, "xh": xh,
            "wqT": wqT, "wkT": wkT, "wvT": wvT,
            "bq": bq, "bk": bk, "bv": bv,
        })
    return in_maps, x


def run_on_hw(inputs, trace=False, **kwargs):
    """Returns (list of per-core attn_part arrays, BassKernelResults)."""
    from concourse import bass_utils
    nc = _get_nc()
    in_maps, _ = _make_in_maps(inputs)
    res = bass_utils.run_bass_kernel_spmd(
        nc, in_maps, list(range(NCORES)), trace=trace, **kwargs)
    parts = [res.results[i]["attn_part"] for i in range(NCORES)]
    return parts, res


def kernel(**inputs) -> np.ndarray:
    in_maps, x = _make_in_maps(inputs)
    parts, _ = run_on_hw(inputs)
    out = np.empty((N, C, H, W), dtype=np.float32)
    for n in range(N):
        attn = parts[2 * n] + parts[2 * n + 1]
        out[n] = x[n] + attn.reshape(C, H, W)
    return out


# revision 7
# speedup vs baseline: 1.4162x; 1.4162x over previous
"""Trainium2 Bass kernel for nn_ContextAttention_21457656611319.

Reference math (per batch n):
    xf = x[n] reshaped [C, L], L = H*W = 4096
    q = Wq@xf + bq ; k = Wk@xf + bk ; v = Wv@xf + bv          [C, L]
    S[l,m] = sum_c k[c,l] q[c,m] * (1/sqrt(C))                 [L, L]
    T = softmax(S, axis=m)  (softmax over the m axis)
    attn[c,m] = sum_l v[c,l] T[l,m]
    out = x + attn

Sharding: 8 cores = 4 batches x 2-way shard of the l (key/value) axis.
Softmax rows (fixed l, all m) stay intact on one core, so each core
computes a partial attn (partial sum over its l-half) independently;
the host adds the two halves per batch plus x.  No collectives needed.

Per-core schedule (l-half LH=2048 -> 16 l-tiles of 128):
  phase 0: q = WqT^T @ xf (+bq) -> bf16 [C, L]
           k = WkT^T @ xh (+bk) -> bf16 [C, LH]
           vT_i = xh_i^T @ WvT (+bv) -> f32 [128, C] per l-tile
  main loop (per l-tile i), all engines pipelined by Tile:
    PE:  S_i = k_i^T @ q in four 1024-col chunks (bf16 in, f32 PSUM,
         2-chunk double buffer in PSUM banks 0-3), plus the previous
         tile's 8 attn matmuls into 4 persistent [128,1024] PSUM
         accumulators (banks 4-7)
    ACT: T_i chunk = exp(scale * S chunk) -> SBUF bf16
    DVE: Z_i = rowsum(T_i) (4 chunk reduces + combine), r = 1/Z,
         vts_i = vT_i * r  (bf16)
  tail: DVE-copy the 4 attn accumulators to SBUF, DMA out (f32).

All matmuls are bf16 (fp32 matmul runs LOW_HIGH double passes on TRN2 =
2x slower); x and the weights are pre-cast to bf16 on the host.
Softmax max-subtraction is skipped: scores*scale ~ N(0,1) here, so
exp() cannot overflow and softmax is shift-invariant anyway.
"""

import sys

if "/opt/trn_rl_repo" not in sys.path:
    sys.path.insert(0, "/opt/trn_rl_repo")

import numpy as np

N, C, H, W = 4, 128, 64, 64
L = H * W            # 4096
LH = L // 2          # 2048 l-half per core
P = 128              # partitions / l-tile size
NT = LH // P         # 16 l-tiles per core
BANK = 512           # fp32 elems per PSUM bank
CH = 1024            # S-chunk / attn-accumulator width (2 PSUM banks)
NCH = L // CH        # 4 chunks
NCORES = 8
SCALE = float(1.0 / np.sqrt(C))

_CACHE = {}


def _build_nc():
    import concourse.bass as bass
    import concourse.tile as tile
    from concourse import bacc, mybir
    from contextlib import ExitStack

    f32 = mybir.dt.float32
    bf16 = mybir.dt.bfloat16

    nc = bacc.Bacc("TRN2", target_bir_lowering=False, debug=False)

    xf = nc.dram_tensor("xf", [P, L], bf16, kind="ExternalInput").ap()
    xh = nc.dram_tensor("xh", [P, LH], bf16, kind="ExternalInput").ap()
    wqT = nc.dram_tensor("wqT", [P, P], bf16, kind="ExternalInput").ap()
    wkT = nc.dram_tensor("wkT", [P, P], bf16, kind="ExternalInput").ap()
    wvT = nc.dram_tensor("wvT", [P, P], bf16, kind="ExternalInput").ap()
    bq = nc.dram_tensor("bq", [P, 1], f32, kind="ExternalInput").ap()
    bk = nc.dram_tensor("bk", [P, 1], f32, kind="ExternalInput").ap()
    bv = nc.dram_tensor("bv", [1, P], f32, kind="ExternalInput").ap()
    attn_out = nc.dram_tensor("attn_part", [P, L], f32, kind="ExternalOutput").ap()

    Exp = mybir.ActivationFunctionType.Exp

    with tile.TileContext(nc) as tc, ExitStack() as ctx:
        const = ctx.enter_context(tc.tile_pool(name="const", bufs=1))
        persist = ctx.enter_context(tc.tile_pool(name="persist", bufs=1))

        wq_sb = const.tile([P, P], bf16)
        wk_sb = const.tile([P, P], bf16)
        wv_sb = const.tile([P, P], bf16)
        bq_sb = const.tile([P, 1], f32)
        bk_sb = const.tile([P, 1], f32)
        bv_sb = const.tile([P, P], f32)  # bv broadcast across partitions
        warm = const.tile([P, 1], f32)
        nc.sync.dma_start(out=wq_sb, in_=wqT)
        nc.sync.dma_start(out=wk_sb, in_=wkT)
        nc.sync.dma_start(out=wv_sb, in_=wvT)
        nc.sync.dma_start(out=bq_sb, in_=bq)
        nc.sync.dma_start(out=bk_sb, in_=bk)
        bv_bcast = bass.AP(tensor=bv.tensor, offset=bv.offset,
                           ap=[[0, P], bv.ap[1]])
        nc.sync.dma_start(out=bv_sb, in_=bv_bcast)
        # warm the ACT exp table while DMAs run (first exp otherwise pays
        # the ~2.7us ACT_TABLE_LOAD on the critical path)
        nc.scalar.activation(warm, bq_sb, Exp, scale=0.0)

        q_sb = persist.tile([P, L], bf16)
        k_sb = persist.tile([P, LH], bf16)
        vt_sb = persist.tile([P, NT, P], f32)   # [l, tile, c]
        vts = persist.tile([P, NT, P], bf16)    # vT * (1/Z), bf16
        zs = persist.tile([P, NT], f32)
        rs = persist.tile([P, NT], f32)
        attn_sb = persist.tile([P, L], f32)     # attn partial accumulator

        # ---- phase 0: projections -------------------------------------
        with tc.tile_pool(name="xp", bufs=1) as xp, \
             tc.tile_pool(name="p0ps", bufs=2, space="PSUM") as p0:
            x_sb = xp.tile([P, L], bf16)
            xh_sb = xp.tile([P, LH], bf16)
            # split the big DMA so downstream matmuls can start early
            nc.sync.dma_start(out=x_sb[:, :LH], in_=xf[:, :LH])
            nc.sync.dma_start(out=x_sb[:, LH:], in_=xf[:, LH:])
            nc.sync.dma_start(out=xh_sb, in_=xh)

            # q = WqT^T @ x + bq, two 2048-wide passes -> bf16
            for h in range(2):
                t = p0.tile([P, LH], f32, tag="p0")
                for j in range(LH // BANK):
                    c0 = h * LH + j * BANK
                    nc.tensor.matmul(t[:, j * BANK:(j + 1) * BANK],
                                     wq_sb, x_sb[:, c0:c0 + BANK])
                nc.vector.tensor_scalar_add(q_sb[:, h * LH:(h + 1) * LH], t, bq_sb)

            # k = WkT^T @ xh + bk -> bf16
            t = p0.tile([P, LH], f32, tag="p0")
            for j in range(LH // BANK):
                nc.tensor.matmul(t[:, j * BANK:(j + 1) * BANK],
                                 wk_sb, xh_sb[:, j * BANK:(j + 1) * BANK])
            nc.vector.tensor_scalar_add(k_sb, t, bk_sb)

            # vT_i = xh_i^T @ WvT (+ bv broadcast along free dim) -> f32
            t = p0.tile([P, LH], f32, tag="p0")
            for i in range(NT):
                nc.tensor.matmul(t[:, i * P:(i + 1) * P],
                                 xh_sb[:, i * P:(i + 1) * P], wv_sb)
            for i in range(NT):
                nc.vector.tensor_add(vt_sb[:, i, :], t[:, i * P:(i + 1) * P], bv_sb)

        # ---- T storage (reuses the SBUF space freed by xp) -------------
        tpool = ctx.enter_context(tc.tile_pool(name="tpool", bufs=1))
        t_all = tpool.tile([P, NT, L], bf16)

        # ---- main loop: scores/softmax + interleaved attn matmuls ------
        # attn is accumulated in PSUM over groups of GRP l-tiles, one
        # 1024-wide m-range sub-pass at a time (2 banks), then flushed
        # into attn_sb by DVE.  Group g's 4 sub-passes are spread across
        # the 4 tiles of group g+1 so PE stays fed while ACT runs exp.
        GRP = 4
        NGRP = NT // GRP
        with tc.tile_pool(name="sps", bufs=2, space="PSUM") as sp, \
             tc.tile_pool(name="aps", bufs=2, space="PSUM") as ap, \
             tc.tile_pool(name="outp", bufs=2) as outp:

            def attn_sub_pass(g, sub):
                t = ap.tile([P, CH], f32, tag="acc", name="acc")
                for idx in range(GRP):
                    i = g * GRP + idx
                    for hh in range(2):
                        m0 = sub * CH + hh * BANK
                        nc.tensor.matmul(t[:, hh * BANK:(hh + 1) * BANK],
                                         vts[:, i, :],
                                         t_all[:, i, m0:m0 + BANK],
                                         start=(idx == 0), stop=(idx == GRP - 1))
                msl = slice(sub * CH, (sub + 1) * CH)
                if g == 0:
                    nc.vector.tensor_copy(attn_sb[:, msl], t)
                elif g < NGRP - 1:
                    nc.vector.tensor_add(attn_sb[:, msl], attn_sb[:, msl], t)
                else:
                    ao = outp.tile([P, CH], f32, tag="ao", name="ao")
                    nc.vector.tensor_add(ao, attn_sb[:, msl], t)
                    nc.sync.dma_start(out=attn_out[:, msl], in_=ao)

            for i in range(NT):
                for c in range(NCH):
                    s = sp.tile([P, CH], f32, tag="s")
                    for j in range(CH // BANK):
                        m0 = c * CH + j * BANK
                        nc.tensor.matmul(s[:, j * BANK:(j + 1) * BANK],
                                         k_sb[:, i * P:(i + 1) * P],
                                         q_sb[:, m0:m0 + BANK])
                    nc.scalar.activation(t_all[:, i, c * CH:(c + 1) * CH], s,
                                         Exp, scale=SCALE)
                nc.vector.reduce_sum(out=zs[:, i:i + 1], in_=t_all[:, i, :],
                                     axis=mybir.AxisListType.X)
                nc.vector.reciprocal(rs[:, i:i + 1], zs[:, i:i + 1])
                nc.vector.tensor_scalar_mul(vts[:, i, :], vt_sb[:, i, :], rs[:, i:i + 1])
                if i >= GRP:
                    attn_sub_pass(i // GRP - 1, i % GRP)
            for sub in range(NCH):
                attn_sub_pass(NGRP - 1, sub)

    nc.compile()
    return nc


def _get_nc():
    if "nc" not in _CACHE:
        _CACHE["nc"] = _build_nc()
    return _CACHE["nc"]


def _make_in_maps(inputs):
    import ml_dtypes
    bf = ml_dtypes.bfloat16
    x = np.ascontiguousarray(np.asarray(inputs["x"], dtype=np.float32))
    wqT = np.ascontiguousarray(np.asarray(inputs["Wq"], dtype=np.float32).T.astype(bf))
    wkT = np.ascontiguousarray(np.asarray(inputs["Wk"], dtype=np.float32).T.astype(bf))
    wvT = np.ascontiguousarray(np.asarray(inputs["Wv"], dtype=np.float32).T.astype(bf))
    bq = np.ascontiguousarray(np.asarray(inputs["bq"], dtype=np.float32).reshape(P, 1))
    bk = np.ascontiguousarray(np.asarray(inputs["bk"], dtype=np.float32).reshape(P, 1))
    bv = np.ascontiguousarray(np.asarray(inputs["bv"], dtype=np.float32).reshape(1, P))
    in_maps = []
    for core in range(NCORES):
        n, half = core // 2, core % 2
        xf32 = x[n].reshape(C, L)
        xfb = np.ascontiguousarray(xf32.astype(bf))
        xhb = np.ascontiguousarray(xfb[:, half * LH:(half + 1) * LH])
        in_maps.append({
            "xf": xfb, "xh": xhb,
            "wqT": wqT, "wkT": wkT, "wvT": wvT,
            "bq": bq, "bk": bk, "bv": bv,
        })
    return in_maps, x


def run_on_hw(inputs, trace=False, **kwargs):
    """Returns (list of per-core attn_part arrays, BassKernelResults)."""
    from concourse import bass_utils
    nc = _get_nc()
    in_maps, _ = _make_in_maps(inputs)
    res = bass_utils.run_bass_kernel_spmd(
        nc, in_maps, list(range(NCORES)), trace=trace, **kwargs)
    parts = [res.results[i]["attn_part"] for i in range(NCORES)]
    return parts, res


def kernel(**inputs) -> np.ndarray:
    in_maps, x = _make_in_maps(inputs)
    parts, _ = run_on_hw(inputs)
    out = np.empty((N, C, H, W), dtype=np.float32)
    for n in range(N):
        attn = parts[2 * n] + parts[2 * n + 1]
        out[n] = x[n] + attn.reshape(C, H, W)
    return out
